# revision 4
# baseline (speedup 1.0000x reference)
"""Expert-parallel MoE FFN kernel for Trainium2 (8 NeuronCores).

Problem: y[e] = relu(x[e] @ w1[e].T) @ w2[e].T for 8 experts.
Sharding: expert-parallel — expert e runs entirely on core e; no
cross-core communication.

Per-core dataflow (x:[2048,1024], w1:[4096,1024], w2:[1024,4096]):
  phase 0: load x natural, PE-transpose 128x128 blocks -> resident xT
  phase 1: stream w1 rows, PE-transpose -> w1T tiles;
           hT[hid,tok] = sum_k w1T[k,hid].T-style matmul accum in PSUM,
           fused ReLU on eviction; spill h[4096,2048] to DRAM scratch
  phase 2: build resident w2T (transpose of w2) in SBUF;
           stream hT token tiles from scratch;
           y[tok,dm] = sum_hid matmul(lhsT=hT_tile, rhs=w2T) -> out

Matmuls run as float32r (full-rate fp32 mode on the PE array); data
stays fp32 end to end.
"""

import sys

if "/opt/trn_rl_repo" not in sys.path:
    sys.path.insert(0, "/opt/trn_rl_repo")

import numpy as np

import concourse.bass as bass  # noqa: F401  (re-exported for callers)
import concourse.mybir as mybir
from concourse import bacc
from concourse.bass_utils import run_bass_kernel_spmd
from concourse.masks import make_identity
from concourse.tile import TileContext

P = 128
TOK = 2048
DM = 1024
DH = 4096
N_CORES = 8

MM_DT = mybir.dt.float32r  # matmul compute mode for fp32 data


def build_nc(mm_dt=MM_DT):
    f32 = mybir.dt.float32
    nc = bacc.Bacc("TRN2", target_bir_lowering=False, debug=False)
    x = nc.dram_tensor("x", [TOK, DM], f32, kind="ExternalInput")
    w1 = nc.dram_tensor("w1", [DH, DM], f32, kind="ExternalInput")
    w2 = nc.dram_tensor("w2", [DM, DH], f32, kind="ExternalInput")
    y = nc.dram_tensor("y", [TOK, DM], f32, kind="ExternalOutput")

    KD = DM // P  # 8 contraction tiles for GEMM1
    KH = DH // P  # 32 contraction tiles for GEMM2
    MT = TOK // P  # 16 token tiles
    NB = TOK // 512  # 4 token blocks (psum free dim 512)
    DB = DM // 512  # 2 dm blocks

    td = mm_dt  # dtype of PE-feeding tiles (producer rounds to fp32r)

    relu = mybir.ActivationFunctionType.Relu
    copyf = mybir.ActivationFunctionType.Copy

    with TileContext(nc) as tc:
        with (
            tc.tile_pool(name="const", bufs=1) as const,
            tc.tile_pool(name="dram", bufs=1, space="DRAM") as dram,
        ):
            ident = const.tile([P, P], f32)
            make_identity(nc, ident)
            h = dram.tile([DH, TOK], td)

            # ---- phase 0: x -> xT (resident) ----
            with (
                tc.tile_pool(name="xT", bufs=1) as xT_pool,
                tc.tile_pool(name="nat", bufs=3) as nat_pool,
                tc.tile_pool(name="w1T", bufs=2) as w1T_pool,
                tc.tile_pool(name="hstage", bufs=3) as h_pool,
                tc.tile_pool(name="tp", bufs=2, space="PSUM") as tps,
                tc.tile_pool(name="mm1", bufs=4, space="PSUM") as mm1,
            ):
                xT = xT_pool.tile([P, KD, TOK], td)
                for mt in range(MT):
                    xa = nat_pool.tile([P, DM], f32, tag="xnat")
                    nc.sync.dma_start(xa[:], x[mt * P : (mt + 1) * P, :])
                    for kt in range(KD):
                        pt = tps.tile([P, P], f32)
                        nc.tensor.transpose(
                            pt[:], xa[:, kt * P : (kt + 1) * P], ident[:]
                        )
                        nc.vector.tensor_copy(xT[:, kt, mt * P : (mt + 1) * P], pt[:])

                # ---- phase 1: GEMM1 + ReLU, h spilled ----
                for ht in range(KH):
                    wa = nat_pool.tile([P, DM], f32, tag="w1nat")
                    nc.sync.dma_start(wa[:], w1[ht * P : (ht + 1) * P, :])
                    w1T = w1T_pool.tile([P, KD, P], td)
                    for kt in range(KD):
                        pt = tps.tile([P, P], f32)
                        nc.tensor.transpose(
                            pt[:], wa[:, kt * P : (kt + 1) * P], ident[:]
                        )
                        nc.vector.tensor_copy(w1T[:, kt, :], pt[:])
                    hs = h_pool.tile([P, TOK], td)
                    pss = [mm1.tile([P, 512], f32, tag="mm1ps", name=f"mm1ps{_i}") for _i in range(NB)]
                    for kt in range(KD):
                        for nb in range(NB):
                            nc.tensor.matmul(
                                pss[nb][:],
                                w1T[:, kt, :],
                                xT[:, kt, nb * 512 : (nb + 1) * 512],
                                start=(kt == 0),
                                stop=(kt == KD - 1),
                            )
                    for nb in range(NB):
                        seg = slice(nb * 512, (nb + 1) * 512)
                        if nb % 2 == 0:
                            nc.scalar.activation(hs[:, seg], pss[nb][:], relu)
                        else:
                            nc.vector.tensor_scalar_max(hs[:, seg], pss[nb][:], 0.0)
                    nc.sync.dma_start(h[ht * P : (ht + 1) * P, :], hs[:])

            # ---- phase 2: w2T resident, stream hT token tiles ----
            hT_view = h[:].rearrange("(ho p) t -> p ho t", p=P)
            with (
                tc.tile_pool(name="w2T", bufs=1) as w2T_pool,
                tc.tile_pool(name="nat2", bufs=2) as nat2_pool,
                tc.tile_pool(name="hT", bufs=2) as hT_pool,
                tc.tile_pool(name="ys", bufs=3) as y_pool,
                tc.tile_pool(name="tp2", bufs=2, space="PSUM") as tps2,
                tc.tile_pool(name="mm2", bufs=4, space="PSUM") as mm2,
            ):
                w2T = w2T_pool.tile([P, KH, DM], td)
                half = DH // 2
                for dt_ in range(KD):
                    for hh in range(2):
                        wa = nat2_pool.tile([P, half], f32, tag="w2nat")
                        nc.sync.dma_start(
                            wa[:],
                            w2[dt_ * P : (dt_ + 1) * P, hh * half : (hh + 1) * half],
                        )
                        for c in range(KH // 2):
                            ht = hh * (KH // 2) + c
                            pt = tps2.tile([P, P], f32)
                            nc.tensor.transpose(
                                pt[:], wa[:, c * P : (c + 1) * P], ident[:]
                            )
                            nc.vector.tensor_copy(
                                w2T[:, ht, dt_ * P : (dt_ + 1) * P], pt[:]
                            )
                for mt in range(MT):
                    hTt = hT_pool.tile([P, KH, P], td)
                    nc.sync.dma_start(hTt[:], hT_view[:, :, mt * P : (mt + 1) * P])
                    ys = y_pool.tile([P, DM], f32)
                    pss = [mm2.tile([P, 512], f32, tag="mm2ps", name=f"mm2ps{_i}") for _i in range(DB)]
                    for ht in range(KH):
                        for db in range(DB):
                            nc.tensor.matmul(
                                pss[db][:],
                                hTt[:, ht, :],
                                w2T[:, ht, db * 512 : (db + 1) * 512],
                                start=(ht == 0),
                                stop=(ht == KH - 1),
                            )
                    for db in range(DB):
                        seg = slice(db * 512, (db + 1) * 512)
                        if db % 2 == 0:
                            nc.scalar.activation(ys[:, seg], pss[db][:], copyf)
                        else:
                            nc.vector.tensor_copy(ys[:, seg], pss[db][:])
                    nc.sync.dma_start(y[mt * P : (mt + 1) * P, :], ys[:])
    nc.compile()
    return nc


_CACHE = {}


def _get_nc():
    if "nc" not in _CACHE:
        _CACHE["nc"] = build_nc()
    return _CACHE["nc"]


def kernel(x, weight1, weight2):
    x = np.asarray(x, dtype=np.float32)
    weight1 = np.asarray(weight1, dtype=np.float32)
    weight2 = np.asarray(weight2, dtype=np.float32)
    assert x.shape == (N_CORES, TOK, DM)
    assert weight1.shape == (N_CORES, DH, DM)
    assert weight2.shape == (N_CORES, DM, DH)

    nc = _get_nc()
    in_maps = [
        {
            "x": np.ascontiguousarray(x[e]),
            "w1": np.ascontiguousarray(weight1[e]),
            "w2": np.ascontiguousarray(weight2[e]),
        }
        for e in range(N_CORES)
    ]
    res = run_bass_kernel_spmd(nc, in_maps, core_ids=list(range(N_CORES)))
    y = np.stack([res.results[e]["y"] for e in range(N_CORES)], axis=0)
    return y.reshape(1, N_CORES, TOK, DM)


# revision 10
# speedup vs baseline: 1.0886x; 1.0886x over previous
"""Expert-parallel MoE FFN kernel for Trainium2 (8 NeuronCores).

Problem: y[e] = relu(x[e] @ w1[e].T) @ w2[e].T for 8 experts.
Sharding: expert-parallel — expert e runs entirely on core e; no
cross-core communication.

Per-core dataflow (x:[2048,1024], w1:[4096,1024], w2:[1024,4096]):
  phase 0: load x natural, PE-transpose 128x128 blocks -> resident xT
  phase 1: stream w1 rows, PE-transpose -> w1T tiles;
           hT[hid,tok] accumulated in PSUM over d_model tiles, fused
           ReLU on eviction; spill h[4096,2048] to DRAM scratch.
           w2T half 0 (dm cols 0:512) is built during this window so
           the transposes run on a HAM-warm PE and phase 2 can start
           immediately.
  phase 2: stream hT token tiles back; y[tok,dm] accumulated over hid
           tiles against resident w2T. w2T half 1 is built during the
           first token tiles (after xT frees its SBUF); those token
           tiles' second halves are finished in a catch-up loop.

Matmuls run as float32r (full-rate fp32 mode on the PE array); data
stays fp32 end to end; fp32->fp32r rounding happens in the PSUM->SBUF
eviction copies.
"""

import sys

if "/opt/trn_rl_repo" not in sys.path:
    sys.path.insert(0, "/opt/trn_rl_repo")

import numpy as np

import concourse.bass as bass  # noqa: F401
import concourse.mybir as mybir
from concourse import bacc
from concourse.bass_utils import run_bass_kernel_spmd
from concourse.masks import make_identity
from concourse.tile import TileContext

P = 128
TOK = 2048
DM = 1024
DH = 4096
N_CORES = 8

MM_DT = mybir.dt.float32r


def build_nc(mm_dt=MM_DT):
    f32 = mybir.dt.float32
    nc = bacc.Bacc("TRN2", target_bir_lowering=False, debug=False)
    x = nc.dram_tensor("x", [TOK, DM], f32, kind="ExternalInput")
    w1 = nc.dram_tensor("w1", [DH, DM], f32, kind="ExternalInput")
    w2 = nc.dram_tensor("w2", [DM, DH], f32, kind="ExternalInput")
    y = nc.dram_tensor("y", [TOK, DM], f32, kind="ExternalOutput")

    KD = DM // P  # 8 dm tiles (GEMM1 contraction)
    KH = DH // P  # 32 hid tiles (GEMM2 contraction)
    MT = TOK // P  # 16 token tiles
    NB = TOK // 512  # 4 token blocks
    td = mm_dt

    relu = mybir.ActivationFunctionType.Relu
    copyf = mybir.ActivationFunctionType.Copy

    with TileContext(nc) as tc:
        with (
            tc.tile_pool(name="const", bufs=1) as const,
            tc.tile_pool(name="dram", bufs=1, space="DRAM") as dram,
            tc.tile_pool(name="w2T0res", bufs=1) as w2T0_pool,
            tc.tile_pool(name="nat", bufs=4) as nat_pool,
            tc.tile_pool(name="tp", bufs=4, space="PSUM") as tps,
            tc.tile_pool(name="mm", bufs=4, space="PSUM") as mmp,
        ):
            ident = const.tile([P, P], f32)
            make_identity(nc, ident)
            h = dram.tile([DH, TOK], td)

            ncopy = [0]  # alternate PSUM->SBUF copy engine

            def evict_copy(dst, src):
                ncopy[0] += 1
                if ncopy[0] % 2 == 0:
                    nc.vector.tensor_copy(dst, src)
                else:
                    nc.scalar.activation(dst, src, copyf)

            def transpose_block(dst, src):
                pt = tps.tile([P, P], f32, name="tp")
                nc.tensor.transpose(pt[:], src, ident[:])
                evict_copy(dst, pt[:])

            # w2T halves: [P, KH, 512] each (dm cols 0:512 / 512:1024).
            # Half 0 lives for the whole kernel; half 1 is created after
            # xT's pool closes so it reuses that SBUF range.
            w2T = [w2T0_pool.tile([P, KH, 512], td, name="w2T0"), None]

            def build_w2_chunk(dt_):
                # transpose w2 rows dt_*128:(dt_+1)*128 into its w2T half
                hb = dt_ // 4
                col = (dt_ % 4) * P
                for q in range(4):
                    wa = nat_pool.tile([P, DM], f32, tag="nat", name="w2a")
                    nc.sync.dma_start(
                        wa[:],
                        w2[dt_ * P : (dt_ + 1) * P, q * DM : (q + 1) * DM],
                    )
                    for c in range(KD):
                        ht = q * KD + c
                        transpose_block(
                            w2T[hb][:, ht, col : col + P],
                            wa[:, c * P : (c + 1) * P],
                        )

            # ---- phases 0+1 inside xT's pool scope ----
            p01 = tc.alloc_tile_pool(name="p01", bufs=1)
            w1T_pool = tc.alloc_tile_pool(name="w1T", bufs=2)
            h_pool = tc.alloc_tile_pool(name="hstage", bufs=3)
            xT = p01.tile([P, KD, TOK], td, name="xT")
            for mt in range(MT):
                xa = nat_pool.tile([P, DM], f32, tag="nat", name="xa")
                nc.sync.dma_start(xa[:], x[mt * P : (mt + 1) * P, :])
                for kt in range(KD):
                    transpose_block(
                        xT[:, kt, mt * P : (mt + 1) * P],
                        xa[:, kt * P : (kt + 1) * P],
                    )

            # ---- phase 1: GEMM1 (+ w2T half 0 woven in) ----
            for ht in range(KH):
                wa = nat_pool.tile([P, DM], f32, tag="nat", name="w1a")
                nc.sync.dma_start(wa[:], w1[ht * P : (ht + 1) * P, :])
                w1T = w1T_pool.tile([P, KD, P], td)
                for kt in range(KD):
                    transpose_block(w1T[:, kt, :], wa[:, kt * P : (kt + 1) * P])
                hs = h_pool.tile([P, TOK], td)
                for nb in range(NB):
                    ps = mmp.tile([P, 512], f32, tag="ps", name="ps1")
                    for kt in range(KD):
                        nc.tensor.matmul(
                            ps[:],
                            w1T[:, kt, :],
                            xT[:, kt, nb * 512 : (nb + 1) * 512],
                            start=(kt == 0),
                            stop=(kt == KD - 1),
                        )
                    seg = slice(nb * 512, (nb + 1) * 512)
                    if nb % 2 == 0:
                        nc.scalar.activation(hs[:, seg], ps[:], relu)
                    else:
                        nc.vector.tensor_scalar_max(hs[:, seg], ps[:], 0.0)
                nc.sync.dma_start(h[ht * P : (ht + 1) * P, :], hs[:])
                if ht % 8 == 7:
                    build_w2_chunk(ht // 8)  # dt_ 0..3 -> w2T half 0

            # ---- phase 2: GEMM2 ----
            h_pool.release()
            w1T_pool.release()
            p01.release()
            w2T1_pool = tc.alloc_tile_pool(name="w2T1res", bufs=1)
            hT_pool = tc.alloc_tile_pool(name="hT", bufs=2)
            y_pool = tc.alloc_tile_pool(name="ys", bufs=4)
            w2T[1] = w2T1_pool.tile([P, KH, 512], td, name="w2T1")
            hT_view = h[:].rearrange("(ho p) t -> p ho t", p=P)

            def gemm2_group(mt, db, hTt):
                ps = mmp.tile([P, 512], f32, tag="ps", name="ps2")
                for ht in range(KH):
                    nc.tensor.matmul(
                        ps[:],
                        hTt[:, ht, :],
                        w2T[db][:, ht, :],
                        start=(ht == 0),
                        stop=(ht == KH - 1),
                    )
                ys = y_pool.tile([P, 512], f32)
                evict_copy(ys[:], ps[:])
                nc.sync.dma_start(
                    y[mt * P : (mt + 1) * P, db * 512 : (db + 1) * 512], ys[:]
                )

            for mt in range(MT):
                hTt = hT_pool.tile([P, KH, P], td)
                nc.sync.dma_start(hTt[:], hT_view[:, :, mt * P : (mt + 1) * P])
                gemm2_group(mt, 0, hTt)
                if mt < 4:
                    # build w2T half 1 while GEMM2 runs (xT space frees now)
                    build_w2_chunk(4 + mt)
                else:
                    gemm2_group(mt, 1, hTt)
            for mt in range(4):  # catch-up: second halves of the first 4 tiles
                hTt = hT_pool.tile([P, KH, P], td)
                nc.sync.dma_start(hTt[:], hT_view[:, :, mt * P : (mt + 1) * P])
                gemm2_group(mt, 1, hTt)
            y_pool.release()
            hT_pool.release()
            w2T1_pool.release()
    nc.compile()
    return nc


_CACHE = {}


def _get_nc():
    if "nc" not in _CACHE:
        _CACHE["nc"] = build_nc()
    return _CACHE["nc"]


def kernel(x, weight1, weight2):
    x = np.asarray(x, dtype=np.float32)
    weight1 = np.asarray(weight1, dtype=np.float32)
    weight2 = np.asarray(weight2, dtype=np.float32)
    assert x.shape == (N_CORES, TOK, DM)
    assert weight1.shape == (N_CORES, DH, DM)
    assert weight2.shape == (N_CORES, DM, DH)

    nc = _get_nc()
    in_maps = [
        {
            "x": np.ascontiguousarray(x[e]),
            "w1": np.ascontiguousarray(weight1[e]),
            "w2": np.ascontiguousarray(weight2[e]),
        }
        for e in range(N_CORES)
    ]
    res = run_bass_kernel_spmd(nc, in_maps, core_ids=list(range(N_CORES)))
    y = np.stack([res.results[e]["y"] for e in range(N_CORES)], axis=0)
    return y.reshape(1, N_CORES, TOK, DM)


# revision 11
# speedup vs baseline: 1.1594x; 1.0651x over previous
"""Expert-parallel MoE FFN kernel for Trainium2 (8 NeuronCores).

Problem: y[e] = relu(x[e] @ w1[e].T) @ w2[e].T for 8 experts.
Sharding: expert-parallel — expert e runs entirely on core e; no
cross-core communication.

Per-core dataflow (x:[2048,1024], w1:[4096,1024], w2:[1024,4096]):
  phase 0: load x natural, PE-transpose 128x128 blocks -> resident xT
  phase 1: stream w1 rows, PE-transpose -> w1T tiles;
           hT[hid,tok] accumulated in PSUM over d_model tiles, fused
           ReLU on eviction; spill h[4096,2048] to DRAM scratch.
           w2T half 0 (dm cols 0:512) is built during this window so
           the transposes run on a HAM-warm PE and phase 2 can start
           immediately.
  phase 2: stream hT token tiles back; y[tok,dm] accumulated over hid
           tiles against resident w2T. w2T half 1 is built during the
           first token tiles (after xT frees its SBUF); those token
           tiles' second halves are finished in a catch-up loop.

Matmuls run as float32r (full-rate fp32 mode on the PE array); data
stays fp32 end to end; fp32->fp32r rounding happens in the PSUM->SBUF
eviction copies.
"""

import sys

if "/opt/trn_rl_repo" not in sys.path:
    sys.path.insert(0, "/opt/trn_rl_repo")

import numpy as np

import concourse.bass as bass  # noqa: F401
import concourse.mybir as mybir
from concourse import bacc
from concourse.bass_utils import run_bass_kernel_spmd
from concourse.masks import make_identity
from concourse.tile import TileContext

P = 128
TOK = 2048
DM = 1024
DH = 4096
N_CORES = 8

MM_DT = mybir.dt.float32r


def build_nc(mm_dt=MM_DT):
    f32 = mybir.dt.float32
    nc = bacc.Bacc("TRN2", target_bir_lowering=False, debug=False)
    x = nc.dram_tensor("x", [TOK, DM], f32, kind="ExternalInput")
    w1 = nc.dram_tensor("w1", [DH, DM], f32, kind="ExternalInput")
    w2 = nc.dram_tensor("w2", [DM, DH], f32, kind="ExternalInput")
    y = nc.dram_tensor("y", [TOK, DM], f32, kind="ExternalOutput")

    KD = DM // P  # 8 dm tiles (GEMM1 contraction)
    KH = DH // P  # 32 hid tiles (GEMM2 contraction)
    MT = TOK // P  # 16 token tiles
    NB = TOK // 512  # 4 token blocks
    td = mm_dt

    relu = mybir.ActivationFunctionType.Relu
    copyf = mybir.ActivationFunctionType.Copy

    with TileContext(nc) as tc:
        with (
            tc.tile_pool(name="const", bufs=1) as const,
            tc.tile_pool(name="dram", bufs=1, space="DRAM") as dram,
            tc.tile_pool(name="w2T0res", bufs=1) as w2T0_pool,
            tc.tile_pool(name="nat", bufs=6) as nat_pool,
            tc.tile_pool(name="tp", bufs=4, space="PSUM") as tps,
            tc.tile_pool(name="mm", bufs=4, space="PSUM") as mmp,
        ):
            ident = const.tile([P, P], f32)
            make_identity(nc, ident)
            h = dram.tile([DH, TOK], td)

            ncopy = [0]  # alternate PSUM->SBUF copy engine

            def evict_copy(dst, src):
                ncopy[0] += 1
                if ncopy[0] % 2 == 0:
                    nc.vector.tensor_copy(dst, src)
                else:
                    nc.scalar.activation(dst, src, copyf)

            def transpose_block(dst, src):
                pt = tps.tile([P, P], f32, name="tp")
                nc.tensor.transpose(pt[:], src, ident[:])
                evict_copy(dst, pt[:])

            # w2T halves: [P, KH, 512] each (dm cols 0:512 / 512:1024).
            # Half 0 lives for the whole kernel; half 1 is created after
            # xT's pool closes so it reuses that SBUF range.
            w2T = [w2T0_pool.tile([P, KH, 512], td, name="w2T0"), None]

            def build_w2_quarter(dt_, q):
                # transpose w2 rows dt_*128:(dt_+1)*128, hid cols q*1024:(q+1)*1024
                hb = dt_ // 4
                col = (dt_ % 4) * P
                wa = nat_pool.tile([P, DM], f32, tag="nat", name="w2a")
                nc.sync.dma_start(
                    wa[:], w2[dt_ * P : (dt_ + 1) * P, q * DM : (q + 1) * DM]
                )
                for c in range(KD):
                    ht = q * KD + c
                    transpose_block(
                        w2T[hb][:, ht, col : col + P], wa[:, c * P : (c + 1) * P]
                    )

            def build_w2_chunk(dt_):
                for q in range(4):
                    build_w2_quarter(dt_, q)

            # ---- phases 0+1 inside xT's pool scope ----
            p01 = tc.alloc_tile_pool(name="p01", bufs=1)
            w1T_pool = tc.alloc_tile_pool(name="w1T", bufs=2)
            h_pool = tc.alloc_tile_pool(name="hstage", bufs=3)
            xT = p01.tile([P, KD, TOK], td, name="xT")
            for mt in range(MT):
                xa = nat_pool.tile([P, DM], f32, tag="nat", name="xa")
                nc.sync.dma_start(xa[:], x[mt * P : (mt + 1) * P, :])
                for kt in range(KD):
                    transpose_block(
                        xT[:, kt, mt * P : (mt + 1) * P],
                        xa[:, kt * P : (kt + 1) * P],
                    )

            # ---- phase 1: GEMM1 (+ w2T half 0 woven in) ----
            for ht in range(KH):
                wa = nat_pool.tile([P, DM], f32, tag="nat", name="w1a")
                nc.sync.dma_start(wa[:], w1[ht * P : (ht + 1) * P, :])
                w1T = w1T_pool.tile([P, KD, P], td)
                for kt in range(KD):
                    transpose_block(w1T[:, kt, :], wa[:, kt * P : (kt + 1) * P])
                hs = h_pool.tile([P, TOK], td)
                for nb in range(NB):
                    ps = mmp.tile([P, 512], f32, tag="ps", name="ps1")
                    for kt in range(KD):
                        nc.tensor.matmul(
                            ps[:],
                            w1T[:, kt, :],
                            xT[:, kt, nb * 512 : (nb + 1) * 512],
                            start=(kt == 0),
                            stop=(kt == KD - 1),
                        )
                    seg = slice(nb * 512, (nb + 1) * 512)
                    if nb % 2 == 0:
                        nc.scalar.activation(hs[:, seg], ps[:], relu)
                    else:
                        nc.vector.tensor_scalar_max(hs[:, seg], ps[:], 0.0)
                nc.sync.dma_start(h[ht * P : (ht + 1) * P, :], hs[:])
                if ht % 2 == 1:
                    qg = ht // 2  # 0..15 -> dt_ 0..3 (w2T half 0)
                    build_w2_quarter(qg // 4, qg % 4)

            # ---- phase 2: GEMM2 ----
            h_pool.release()
            w1T_pool.release()
            p01.release()
            w2T1_pool = tc.alloc_tile_pool(name="w2T1res", bufs=1)
            hT_pool = tc.alloc_tile_pool(name="hT", bufs=2)
            y_pool = tc.alloc_tile_pool(name="ys", bufs=4)
            w2T[1] = w2T1_pool.tile([P, KH, 512], td, name="w2T1")
            hT_view = h[:].rearrange("(ho p) t -> p ho t", p=P)

            def gemm2_group(mt, db, hTt):
                ps = mmp.tile([P, 512], f32, tag="ps", name="ps2")
                for ht in range(KH):
                    nc.tensor.matmul(
                        ps[:],
                        hTt[:, ht, :],
                        w2T[db][:, ht, :],
                        start=(ht == 0),
                        stop=(ht == KH - 1),
                    )
                ys = y_pool.tile([P, 512], f32)
                evict_copy(ys[:], ps[:])
                nc.sync.dma_start(
                    y[mt * P : (mt + 1) * P, db * 512 : (db + 1) * 512], ys[:]
                )

            def load_hT(mt):
                hTt = hT_pool.tile([P, KH, P], td, name="hTt")
                for hq in range(4):
                    nc.sync.dma_start(
                        hTt[:, hq * 8 : (hq + 1) * 8, :],
                        hT_view[:, hq * 8 : (hq + 1) * 8, mt * P : (mt + 1) * P],
                    )
                return hTt

            for mt in range(MT):
                if mt < 4:
                    # build w2T half 1 while GEMM2 starts (xT space frees now)
                    build_w2_chunk(4 + mt)
                hTt = load_hT(mt)
                gemm2_group(mt, 0, hTt)
                if mt >= 4:
                    gemm2_group(mt, 1, hTt)
            for mt in range(4):  # catch-up: second halves of the first 4 tiles
                hTt = load_hT(mt)
                gemm2_group(mt, 1, hTt)
            y_pool.release()
            hT_pool.release()
            w2T1_pool.release()
    nc.compile()
    return nc


_CACHE = {}


def _get_nc():
    if "nc" not in _CACHE:
        _CACHE["nc"] = build_nc()
    return _CACHE["nc"]


def kernel(x, weight1, weight2):
    x = np.asarray(x, dtype=np.float32)
    weight1 = np.asarray(weight1, dtype=np.float32)
    weight2 = np.asarray(weight2, dtype=np.float32)
    assert x.shape == (N_CORES, TOK, DM)
    assert weight1.shape == (N_CORES, DH, DM)
    assert weight2.shape == (N_CORES, DM, DH)

    nc = _get_nc()
    in_maps = [
        {
            "x": np.ascontiguousarray(x[e]),
            "w1": np.ascontiguousarray(weight1[e]),
            "w2": np.ascontiguousarray(weight2[e]),
        }
        for e in range(N_CORES)
    ]
    res = run_bass_kernel_spmd(nc, in_maps, core_ids=list(range(N_CORES)))
    y = np.stack([res.results[e]["y"] for e in range(N_CORES)], axis=0)
    return y.reshape(1, N_CORES, TOK, DM)


# revision 13
# speedup vs baseline: 1.1617x; 1.0020x over previous
"""Expert-parallel MoE FFN kernel for Trainium2 (8 NeuronCores).

Problem: y[e] = relu(x[e] @ w1[e].T) @ w2[e].T for 8 experts.
Sharding: expert-parallel — expert e runs entirely on core e; no
cross-core communication.

Per-core dataflow (x:[2048,1024], w1:[4096,1024], w2:[1024,4096]):
  phase 0: load x natural, PE-transpose 128x128 blocks -> resident xT
  phase 1: stream w1 rows, PE-transpose -> w1T tiles;
           hT[hid,tok] accumulated in PSUM over d_model tiles, fused
           ReLU on eviction; spill h[4096,2048] to DRAM scratch.
           w2T half 0 (dm cols 0:512) is built during this window so
           the transposes run on a HAM-warm PE and phase 2 can start
           immediately.
  phase 2: stream hT token tiles back; y[tok,dm] accumulated over hid
           tiles against resident w2T. w2T half 1 is built during the
           first token tiles (after xT frees its SBUF); those token
           tiles' second halves are finished in a catch-up loop.

Matmuls run as float32r (full-rate fp32 mode on the PE array); data
stays fp32 end to end; fp32->fp32r rounding happens in the PSUM->SBUF
eviction copies.
"""

import sys

if "/opt/trn_rl_repo" not in sys.path:
    sys.path.insert(0, "/opt/trn_rl_repo")

import numpy as np

import concourse.bass as bass  # noqa: F401
import concourse.mybir as mybir
from concourse import bacc
from concourse.bass_utils import run_bass_kernel_spmd
from concourse.masks import make_identity
from concourse.tile import TileContext

P = 128
TOK = 2048
DM = 1024
DH = 4096
N_CORES = 8

MM_DT = mybir.dt.float32r


def build_nc(mm_dt=MM_DT):
    f32 = mybir.dt.float32
    nc = bacc.Bacc("TRN2", target_bir_lowering=False, debug=False)
    x = nc.dram_tensor("x", [TOK, DM], f32, kind="ExternalInput")
    w1 = nc.dram_tensor("w1", [DH, DM], f32, kind="ExternalInput")
    w2 = nc.dram_tensor("w2", [DM, DH], f32, kind="ExternalInput")
    y = nc.dram_tensor("y", [TOK, DM], f32, kind="ExternalOutput")

    KD = DM // P  # 8 dm tiles (GEMM1 contraction)
    KH = DH // P  # 32 hid tiles (GEMM2 contraction)
    MT = TOK // P  # 16 token tiles
    NB = TOK // 512  # 4 token blocks
    td = mm_dt

    relu = mybir.ActivationFunctionType.Relu
    copyf = mybir.ActivationFunctionType.Copy

    with TileContext(nc) as tc:
        with (
            tc.tile_pool(name="const", bufs=1) as const,
            tc.tile_pool(name="dram", bufs=1, space="DRAM") as dram,
            tc.tile_pool(name="w2T0res", bufs=1) as w2T0_pool,
            tc.tile_pool(name="nat", bufs=6) as nat_pool,
            tc.tile_pool(name="tp", bufs=4, space="PSUM") as tps,
            tc.tile_pool(name="mm", bufs=4, space="PSUM") as mmp,
        ):
            ident = const.tile([P, P], f32)
            make_identity(nc, ident)
            h = dram.tile([DH, TOK], td)

            ncopy = [0]  # alternate PSUM->SBUF copy engine

            def evict_copy(dst, src):
                ncopy[0] += 1
                if ncopy[0] % 2 == 0:
                    nc.vector.tensor_copy(dst, src)
                else:
                    nc.scalar.activation(dst, src, copyf)

            def transpose_block(dst, src):
                pt = tps.tile([P, P], f32, name="tp")
                nc.tensor.transpose(pt[:], src, ident[:])
                evict_copy(dst, pt[:])

            # w2T halves: [P, KH, 512] each (dm cols 0:512 / 512:1024).
            # Half 0 lives for the whole kernel; half 1 is created after
            # xT's pool closes so it reuses that SBUF range.
            w2T = [w2T0_pool.tile([P, KH, 512], td, name="w2T0"), None]

            def build_w2_quarter(dt_, q):
                # transpose w2 rows dt_*128:(dt_+1)*128, hid cols q*1024:(q+1)*1024
                hb = dt_ // 4
                col = (dt_ % 4) * P
                wa = nat_pool.tile([P, DM], f32, tag="nat", name="w2a")
                nc.sync.dma_start(
                    wa[:], w2[dt_ * P : (dt_ + 1) * P, q * DM : (q + 1) * DM]
                )
                for c in range(KD):
                    ht = q * KD + c
                    transpose_block(
                        w2T[hb][:, ht, col : col + P], wa[:, c * P : (c + 1) * P]
                    )

            def build_w2_chunk(dt_):
                for q in range(4):
                    build_w2_quarter(dt_, q)

            # ---- phases 0+1 inside xT's pool scope ----
            p01 = tc.alloc_tile_pool(name="p01", bufs=1)
            w1T_pool = tc.alloc_tile_pool(name="w1T", bufs=2)
            h_pool = tc.alloc_tile_pool(name="hstage", bufs=3)
            xT = p01.tile([P, KD, TOK], td, name="xT")
            for mt in range(MT):
                xa = nat_pool.tile([P, DM], f32, tag="nat", name="xa")
                nc.sync.dma_start(xa[:], x[mt * P : (mt + 1) * P, :])
                for kt in range(KD):
                    transpose_block(
                        xT[:, kt, mt * P : (mt + 1) * P],
                        xa[:, kt * P : (kt + 1) * P],
                    )

            # ---- phase 1: GEMM1 (+ w2T half 0 woven in) ----
            for ht in range(KH):
                wa = nat_pool.tile([P, DM], f32, tag="nat", name="w1a")
                nc.sync.dma_start(wa[:], w1[ht * P : (ht + 1) * P, :])
                w1T = w1T_pool.tile([P, KD, P], td)
                for kt in range(KD):
                    transpose_block(w1T[:, kt, :], wa[:, kt * P : (kt + 1) * P])
                hs = h_pool.tile([P, TOK], td)
                for nb in range(NB):
                    ps = mmp.tile([P, 512], f32, tag="ps", name="ps1")
                    for kt in range(KD):
                        nc.tensor.matmul(
                            ps[:],
                            w1T[:, kt, :],
                            xT[:, kt, nb * 512 : (nb + 1) * 512],
                            start=(kt == 0),
                            stop=(kt == KD - 1),
                        )
                    seg = slice(nb * 512, (nb + 1) * 512)
                    if nb % 2 == 0:
                        nc.scalar.activation(hs[:, seg], ps[:], relu)
                    else:
                        nc.vector.tensor_scalar_max(hs[:, seg], ps[:], 0.0)
                nc.sync.dma_start(h[ht * P : (ht + 1) * P, :], hs[:])
                if ht % 2 == 1:
                    qg = ht // 2  # 0..15 -> dt_ 0..3 (w2T half 0)
                    build_w2_quarter(qg // 4, qg % 4)

            # ---- phase 2: GEMM2 ----
            h_pool.release()
            w1T_pool.release()
            p01.release()
            w2T1_pool = tc.alloc_tile_pool(name="w2T1res", bufs=1)
            hT_pool = tc.alloc_tile_pool(name="hT", bufs=2)
            y_pool = tc.alloc_tile_pool(name="ys", bufs=4)
            w2T[1] = w2T1_pool.tile([P, KH, 512], td, name="w2T1")
            hT_view = h[:].rearrange("(ho p) t -> p ho t", p=P)

            def gemm2_group(mt, db, hTt):
                ps = mmp.tile([P, 512], f32, tag="ps", name="ps2")
                for ht in range(KH):
                    nc.tensor.matmul(
                        ps[:],
                        hTt[:, ht, :],
                        w2T[db][:, ht, :],
                        start=(ht == 0),
                        stop=(ht == KH - 1),
                    )
                ys = y_pool.tile([P, 512], f32)
                evict_copy(ys[:], ps[:])
                nc.sync.dma_start(
                    y[mt * P : (mt + 1) * P, db * 512 : (db + 1) * 512], ys[:]
                )

            def load_hT(mt):
                hTt = hT_pool.tile([P, KH, P], td, name="hTt")
                for hq in range(4):
                    nc.sync.dma_start(
                        hTt[:, hq * 8 : (hq + 1) * 8, :],
                        hT_view[:, hq * 8 : (hq + 1) * 8, mt * P : (mt + 1) * P],
                    )
                return hTt

            for mt in range(MT):
                hTt = load_hT(mt)
                if mt < 4:
                    # build w2T half 1 while GEMM2 starts (xT space frees now)
                    build_w2_chunk(4 + mt)
                gemm2_group(mt, 0, hTt)
                if mt >= 4:
                    gemm2_group(mt, 1, hTt)
            for mt in range(4):  # catch-up: second halves of the first 4 tiles
                hTt = load_hT(mt)
                gemm2_group(mt, 1, hTt)
            y_pool.release()
            hT_pool.release()
            w2T1_pool.release()
    nc.compile()
    return nc


_CACHE = {}


def _get_nc():
    if "nc" not in _CACHE:
        _CACHE["nc"] = build_nc()
    return _CACHE["nc"]


def kernel(x, weight1, weight2):
    x = np.asarray(x, dtype=np.float32)
    weight1 = np.asarray(weight1, dtype=np.float32)
    weight2 = np.asarray(weight2, dtype=np.float32)
    assert x.shape == (N_CORES, TOK, DM)
    assert weight1.shape == (N_CORES, DH, DM)
    assert weight2.shape == (N_CORES, DM, DH)

    nc = _get_nc()
    in_maps = [
        {
            "x": np.ascontiguousarray(x[e]),
            "w1": np.ascontiguousarray(weight1[e]),
            "w2": np.ascontiguousarray(weight2[e]),
        }
        for e in range(N_CORES)
    ]
    res = run_bass_kernel_spmd(nc, in_maps, core_ids=list(range(N_CORES)))
    y = np.stack([res.results[e]["y"] for e in range(N_CORES)], axis=0)
    return y.reshape(1, N_CORES, TOK, DM)
